# revision 16
# baseline (speedup 1.0000x reference)
"""DistanceFlowedAttention TRN2 kernel — 8-core head-sharded (tensor parallel).

Math (per reference):
    coef = sigmoid(inter_info @ We_w + We_b) + 1
    Q = coef * (input_Q @ W_Q);  K = coef * (input_K @ W_K);  V = input_V @ W_V
    scores = dist * (Q K^T / 8);  scores[mask] = -1e10
    attn = softmax(scores);  out = (attn @ V) @ W_fc + input_Q
    returns (out, attn)

Sharding: core c owns heads [2c, 2c+1] → columns [128c, 128c+128) of the
QKV projections and rows [128c, 128c+128) of W_fc. dist/mask replicated.
Each core emits its attn slice (bf16) and a full-width partial of the
output (summed + residual-added on host).

Device pipeline per core:
  P1: projections in transposed orientation ([cols, tok]) for Q/K/coef,
      natural ([tok, cols]) for V. Gating fused as one DVE op
      (coef+1)*proj via scalar_tensor_tensor.
  P2: per (batch, q-tile): scores = I@a_pre + Qc^T Kc (mask additive is
      injected by an identity matmul accumulating into the same PSUM);
      t = psum * dist (1 DVE op); e,rowsum = exp via ACT with fused
      accumulate; attn = e * (1/rowsum); PE-transpose of bf16 attn;
      context^T accumulated as V^T @ attn^T.
  P3: out_partial = context^T.T @ W_fc_rows.
"""

import json

import ml_dtypes
import numpy as np

import concourse.bass as bass
import concourse.tile as tile
from concourse import mybir
from concourse.bass_utils import run_bass_kernel_spmd
from concourse.masks import make_identity

B, S, D_MODEL = 2, 2048, 1024
N_HEADS, D_HEAD = 16, 64
N_CORES = 8
HPC = N_HEADS // N_CORES      # heads per core (2)
CPC = HPC * D_HEAD            # projection columns per core (128)
DCH = D_MODEL // 128          # contraction chunks (8)
QT_TILES = S // 128           # q tiles per batch (16)
TCH = S // 512                # 512-wide chunks of t (4)

BF16 = ml_dtypes.bfloat16
F32 = mybir.dt.float32
BF = mybir.dt.bfloat16

TRACE = False                 # test.py may flip this for profiling runs
TRACE_KW = {}


def _split_waits(bir_bytes: bytes, cap: int = 1) -> bytes:
    """This container's walrus rejects >1 sem-wait per instruction.
    Hoist extra waits onto preceding wait-only EventSemaphore insts."""
    bj = json.loads(bir_bytes)
    uid = [0]
    for fn in bj.get("functions", []):
        for blk in fn.get("blocks", []):
            new_insts = []
            for inst in blk.get("instructions", []):
                si = inst.get("sync_info") or {}
                waits = si.get("on_wait") or []
                if len(waits) > cap:
                    keep, extra = waits[:cap], waits[cap:]
                    for i in range(0, len(extra), cap):
                        uid[0] += 1
                        new_insts.append({
                            "debug": inst.get("debug", 0),
                            "engine": inst["engine"],
                            "ins": [], "outs": [],
                            "name": f"waitsplit_{uid[0]}",
                            "opcode": "NoOp",
                            "sync_info": {"on_update": [],
                                          "on_wait": extra[i:i + cap]},
                        })
                    si["on_wait"] = keep
                new_insts.append(inst)
            blk["instructions"] = new_insts
    return json.dumps(bj).encode()


def build_nc():
    nc = bass.Bass("TRN2", target_bir_lowering=False, debug=False,
                   num_devices=N_CORES)
    _orig = nc.to_json_bytes
    nc.to_json_bytes = lambda: _split_waits(_orig())

    dt_in = {}
    for nm in ("xtq", "xtk", "xtv", "xti"):
        dt_in[nm] = nc.dram_tensor(nm, [B, D_MODEL, S], BF, kind="ExternalInput")
    wq = nc.dram_tensor("wq", [D_MODEL, CPC], BF, kind="ExternalInput")
    wk = nc.dram_tensor("wk", [D_MODEL, CPC], BF, kind="ExternalInput")
    wv = nc.dram_tensor("wv", [D_MODEL, CPC], BF, kind="ExternalInput")
    wew = nc.dram_tensor("wew", [D_MODEL, CPC], BF, kind="ExternalInput")
    web = nc.dram_tensor("web", [CPC, 1], F32, kind="ExternalInput")
    wfc = nc.dram_tensor("wfc", [CPC, D_MODEL], BF, kind="ExternalInput")
    dist = nc.dram_tensor("dist", [B, S, S], F32, kind="ExternalInput")
    apre = nc.dram_tensor("apre", [B, S, S], BF, kind="ExternalInput")
    attn_o = nc.dram_tensor("attn", [B, HPC, S, S], BF, kind="ExternalOutput")
    out_o = nc.dram_tensor("out", [B, S, D_MODEL], BF, kind="ExternalOutput")

    with tile.TileContext(nc) as tc:
        with tc.tile_pool(name="persist", bufs=1) as persist, \
             tc.tile_pool(name="stage", bufs=6) as stage, \
             tc.tile_pool(name="coefp", bufs=2) as coefp, \
             tc.tile_pool(name="dists", bufs=2) as dists, \
             tc.tile_pool(name="apres", bufs=2) as apres, \
             tc.tile_pool(name="tsbp", bufs=2) as tsbp, \
             tc.tile_pool(name="ep", bufs=2) as ep, \
             tc.tile_pool(name="attnp", bufs=2) as attnp, \
             tc.tile_pool(name="eTp", bufs=2) as eTp, \
             tc.tile_pool(name="outp", bufs=2) as outp, \
             tc.tile_pool(name="smalls", bufs=4) as smalls, \
             tc.tile_pool(name="psA", bufs=2, space="PSUM") as psA, \
             tc.tile_pool(name="psB", bufs=2, space="PSUM") as psB, \
             tc.tile_pool(name="psC", bufs=2, space="PSUM") as psC:

            ident = persist.tile([128, 128], BF, name="ident")
            make_identity(nc, ident[:])

            # weight slices resident in SBUF
            w_sb = {}
            for nm, dr in (("wq", wq), ("wk", wk), ("wv", wv), ("wew", wew)):
                t = persist.tile([128, DCH, CPC], BF, name=f"{nm}_sb")
                nc.sync.dma_start(
                    t[:], dr.ap().rearrange("(po pi) c -> pi po c", pi=128))
                w_sb[nm] = t
            wfc_sb = persist.tile([CPC, D_MODEL], BF, name="wfc_sb")
            nc.sync.dma_start(wfc_sb[:], wfc.ap()[:])
            web_sb = persist.tile([CPC, 1], F32, name="web_sb")
            nc.sync.dma_start(web_sb[:], web.ap()[:])

            qtg = [persist.tile([CPC, S], BF, name=f"qtg{b}") for b in range(B)]
            ktg = [persist.tile([CPC, S], BF, name=f"ktg{b}") for b in range(B)]
            v_sb = [persist.tile([128, QT_TILES, CPC], BF, name=f"v{b}")
                    for b in range(B)]
            ctxT = [persist.tile([CPC, S], BF, name=f"ctxT{b}") for b in range(B)]

            # ---------- Phase 1: projections ----------
            for b in range(B):
                for tcn in range(TCH):
                    ts = slice(tcn * 512, (tcn + 1) * 512)
                    xst = {}
                    for nm in ("xti", "xtq", "xtk", "xtv"):
                        t = stage.tile([128, DCH, 512], BF, tag="stage")
                        nc.sync.dma_start(
                            t[:], dt_in[nm].ap()[b, :, ts].rearrange(
                                "(po pi) s -> pi po s", pi=128))
                        xst[nm] = t
                    # coef^T tile
                    pc = psB.tile([128, 512], F32, tag="psB")
                    for d in range(DCH):
                        nc.tensor.matmul(pc[:], w_sb["wew"][:, d, :],
                                         xst["xti"][:, d, :],
                                         start=(d == 0), stop=(d == DCH - 1))
                    coefT = coefp.tile([128, 512], BF, tag="coefT")
                    nc.scalar.activation(coefT[:], pc[:],
                                         mybir.ActivationFunctionType.Sigmoid,
                                         bias=web_sb[:], scale=1.0)
                    # Q^T, K^T gated tiles
                    for w_nm, x_nm, dst in (("wq", "xtq", qtg[b]),
                                            ("wk", "xtk", ktg[b])):
                        pq = psB.tile([128, 512], F32, tag="psB")
                        for d in range(DCH):
                            nc.tensor.matmul(pq[:], w_sb[w_nm][:, d, :],
                                             xst[x_nm][:, d, :],
                                             start=(d == 0), stop=(d == DCH - 1))
                        nc.vector.scalar_tensor_tensor(
                            dst[:, ts], coefT[:], 1.0, pq[:],
                            op0=mybir.AluOpType.add, op1=mybir.AluOpType.mult)
                    # V natural tiles
                    for j in range(4):
                        pv = psC.tile([128, CPC], F32, tag="psC")
                        js = slice(j * 128, (j + 1) * 128)
                        for d in range(DCH):
                            nc.tensor.matmul(pv[:], xst["xtv"][:, d, js],
                                             w_sb["wv"][:, d, :],
                                             start=(d == 0), stop=(d == DCH - 1))
                        nc.vector.tensor_copy(v_sb[b][:, tcn * 4 + j, :], pv[:])

            # ---------- Phase 2: attention ----------
            for b in range(B):
                for qg in range(QT_TILES // 4):
                    eTs = [eTp.tile([128, QT_TILES, 512], BF, tag="eT",
                                    name=f"eT_{b}_{qg}_{hl}")
                           for hl in range(HPC)]
                    for qj in range(4):
                        qi = qg * 4 + qj
                        qs = slice(qi * 128, (qi + 1) * 128)
                        qjs = slice(qj * 128, (qj + 1) * 128)
                        dist_t = dists.tile([128, S], F32, tag="dist")
                        nc.sync.dma_start(dist_t[:], dist.ap()[b, qs, :])
                        apre_t = apres.tile([128, S], BF, tag="apre")
                        nc.sync.dma_start(apre_t[:], apre.ap()[b, qs, :])
                        for hl in range(HPC):
                            hs = slice(hl * D_HEAD, (hl + 1) * D_HEAD)
                            tsb = tsbp.tile([128, S], F32, tag="tsb")
                            for th in range(2):
                                pss = psA.tile([128, 1024], F32, tag="psA")
                                for u in range(2):
                                    sl = slice(th * 1024 + u * 512,
                                               th * 1024 + (u + 1) * 512)
                                    psl = pss[:, u * 512:(u + 1) * 512]
                                    nc.tensor.matmul(psl, ident[:],
                                                     apre_t[:, sl],
                                                     start=True, stop=False)
                                    nc.tensor.matmul(psl, qtg[b][hs, qs],
                                                     ktg[b][hs, sl],
                                                     start=False, stop=True)
                                nc.vector.scalar_tensor_tensor(
                                    tsb[:, th * 1024:(th + 1) * 1024], pss[:],
                                    1.0,
                                    dist_t[:, th * 1024:(th + 1) * 1024],
                                    op0=mybir.AluOpType.mult,
                                    op1=mybir.AluOpType.mult)
                            e_t = ep.tile([128, S], BF, tag="e")
                            rowsum = smalls.tile([128, 1], F32, tag="rowsum")
                            nc.scalar.activation(
                                e_t[:], tsb[:],
                                mybir.ActivationFunctionType.Exp,
                                accum_out=rowsum[:])
                            rinv = smalls.tile([128, 1], F32, tag="rinv")
                            nc.vector.reciprocal(rinv[:], rowsum[:])
                            at = attnp.tile([128, S], BF, tag="attn")
                            nc.vector.tensor_scalar_mul(at[:], e_t[:], rinv[:])
                            nc.sync.dma_start(attn_o.ap()[b, hl, qs, :], at[:])
                            # transpose attn (bf16) into this group's eT;
                            # alternate the PSUM->SBUF copies DVE/ACT
                            for g in range(2):
                                pst = psB.tile([128, 1024], BF, tag="psB")
                                for u in range(8):
                                    c = 8 * g + u
                                    nc.tensor.transpose(
                                        pst[:, u * 128:(u + 1) * 128],
                                        at[:, c * 128:(c + 1) * 128], ident[:])
                                dst = eTs[hl][:, 8 * g:8 * (g + 1), qjs]
                                src = pst[:].rearrange("p (c q) -> p c q", q=128)
                                if (g + hl) % 2:
                                    nc.vector.tensor_copy(dst, src)
                                else:
                                    nc.scalar.copy(dst, src)
                    # context^T += V^T attn^T over the 512-wide q group
                    for hl in range(HPC):
                        hs = slice(hl * D_HEAD, (hl + 1) * D_HEAD)
                        pctx = psC.tile([D_HEAD, 512], F32, tag="psC")
                        for c in range(QT_TILES):
                            nc.tensor.matmul(pctx[:], v_sb[b][:, c, hs],
                                             eTs[hl][:, c, :],
                                             start=(c == 0),
                                             stop=(c == QT_TILES - 1))
                        nc.vector.tensor_copy(
                            ctxT[b][hs, qg * 512:(qg + 1) * 512], pctx[:])
                    # fc for this q-group's tokens (overlaps later groups)
                    for ti in range(qg * 4, (qg + 1) * 4):
                        tsl = slice(ti * 128, (ti + 1) * 128)
                        osb = outp.tile([128, D_MODEL], BF, tag="osb")
                        for half in range(2):
                            osl = slice(half * 512, (half + 1) * 512)
                            pfc = psB.tile([128, 512], F32, tag="psB",
                                           name=f"pfc_{b}_{ti}_{half}")
                            nc.tensor.matmul(pfc[:], ctxT[b][:, tsl],
                                             wfc_sb[:, osl],
                                             start=True, stop=True)
                            nc.scalar.copy(osb[:, osl], pfc[:])
                        nc.sync.dma_start(out_o.ap()[b, tsl, :], osb[:])
    return nc


def kernel(input_Q, input_K, input_V, inter_info, dist_factor, attn_mask,
           W_Q, W_K, W_V, W_fc, We_w, We_b):
    input_Q = np.asarray(input_Q, dtype=np.float32)
    input_K = np.asarray(input_K, dtype=np.float32)
    input_V = np.asarray(input_V, dtype=np.float32)
    inter_info = np.asarray(inter_info, dtype=np.float32)
    dist_factor = np.asarray(dist_factor, dtype=np.float32)
    attn_mask = np.asarray(attn_mask)
    W_Q = np.asarray(W_Q, dtype=np.float32)
    W_K = np.asarray(W_K, dtype=np.float32)
    W_V = np.asarray(W_V, dtype=np.float32)
    W_fc = np.asarray(W_fc, dtype=np.float32)
    We_w = np.asarray(We_w, dtype=np.float32)
    We_b = np.asarray(We_b, dtype=np.float32)

    # host prep: transposed bf16 activations, mask additive, weight slices
    def xt(x):
        return np.ascontiguousarray(x.transpose(0, 2, 1)).astype(BF16)

    xtq, xtk, xtv, xti = xt(input_Q), xt(input_K), xt(input_V), xt(inter_info)
    d_safe = np.maximum(dist_factor, np.float32(1e-30))
    a_pre = np.where(attn_mask,
                     np.maximum(np.float32(-1e10) / d_safe, np.float32(-1e38)),
                     np.float32(0.0)).astype(BF16)

    nc = build_nc()
    in_maps = []
    for c in range(N_CORES):
        cs = slice(c * CPC, (c + 1) * CPC)
        in_maps.append({
            "xtq": xtq, "xtk": xtk, "xtv": xtv, "xti": xti,
            "wq": np.ascontiguousarray(W_Q[:, cs] / 8.0).astype(BF16),
            "wk": np.ascontiguousarray(W_K[:, cs]).astype(BF16),
            "wv": np.ascontiguousarray(W_V[:, cs]).astype(BF16),
            "wew": np.ascontiguousarray(We_w[:, cs]).astype(BF16),
            "web": np.ascontiguousarray(We_b[cs]).astype(np.float32).reshape(CPC, 1),
            "wfc": np.ascontiguousarray(W_fc[cs, :]).astype(BF16),
            "dist": dist_factor, "apre": a_pre,
        })

    if TRACE:
        results, exec_ns = _run_and_bench(nc, in_maps, N_CORES)
        kernel.last_exec_ns = exec_ns
    else:
        res = run_bass_kernel_spmd(nc, in_maps, core_ids=list(range(N_CORES)))
        results = res.results

    out = input_Q.astype(np.float32).copy()
    attn = np.empty((B, N_HEADS, S, S), dtype=np.float32)
    for c in range(N_CORES):
        r = results[c]
        out += r["out"].astype(np.float32)
        attn[:, c * HPC:(c + 1) * HPC] = r["attn"].astype(np.float32)
    return out, attn


def _run_and_bench(nc, in_maps, n_cores, iters=12):
    """Replicates bass2jax.run_bass_via_pjrt's multi-core path without
    output-buffer donation, so the compiled executable can be re-invoked
    for wall-clock timing (no NTFF hook is available under this axon
    client, so device-side profiling is not an option)."""
    import time

    import jax
    from jax.experimental.shard_map import shard_map
    from jax.sharding import Mesh, NamedSharding, PartitionSpec

    from concourse import bass2jax, mybir as _mybir

    bass2jax.install_neuronx_cc_hook()
    partition_name = (nc.partition_id_tensor.name
                      if nc.partition_id_tensor else None)
    in_names, out_names, out_avals, zero_outs = [], [], [], []
    for alloc in nc.m.functions[0].allocations:
        if not isinstance(alloc, _mybir.MemoryLocationSet):
            continue
        name = alloc.memorylocations[0].name
        if alloc.kind == "ExternalInput":
            if name != partition_name:
                in_names.append(name)
        elif alloc.kind == "ExternalOutput":
            shape = tuple(alloc.tensor_shape)
            dtype = _mybir.dt.np(alloc.dtype)
            out_names.append(name)
            out_avals.append(jax.core.ShapedArray(shape, dtype))
            zero_outs.append(np.zeros(shape, dtype))
    n_params = len(in_names)
    in_names = in_names + out_names
    if partition_name is not None:
        in_names.append(partition_name)

    def _body(*args):
        operands = list(args)
        if partition_name is not None:
            operands.append(bass2jax.partition_id_tensor())
        return tuple(bass2jax._bass_exec_p.bind(
            *operands,
            out_avals=tuple(out_avals),
            in_names=tuple(in_names),
            out_names=tuple(out_names),
            lowering_input_output_aliases=(),
            sim_require_finite=True,
            sim_require_nnan=True,
            nc=nc,
        ))

    devices = jax.devices()[:n_cores]
    mesh = Mesh(np.asarray(devices), ("core",))
    spec = PartitionSpec("core")
    sharded = jax.jit(
        shard_map(_body, mesh=mesh,
                  in_specs=(spec,) * (n_params + len(out_names)),
                  out_specs=(spec,) * len(out_names), check_rep=False),
        keep_unused=True)

    concat = [np.concatenate([np.asarray(in_maps[c][nm])
                              for c in range(n_cores)], axis=0)
              for nm in in_names[:n_params]]
    concat += [np.zeros((n_cores * z.shape[0], *z.shape[1:]), z.dtype)
               for z in zero_outs]
    sh = NamedSharding(mesh, spec)
    dev_in = [jax.device_put(x, sh) for x in concat]

    out_arrs = sharded(*dev_in)
    jax.block_until_ready(out_arrs)

    # timed repeats (async dispatch, single barrier at the end); per-batch
    # fixed dispatch cost is large under axon, so report the marginal slope
    times = {}
    for nit in (4, 4 + iters):
        t0 = time.perf_counter()
        keep = [sharded(*dev_in) for _ in range(nit)]
        jax.block_until_ready(keep)
        dt = time.perf_counter() - t0
        times[nit] = dt
        print(f"bench: {nit} iters in {dt*1e3:.2f} ms -> "
              f"{dt/nit*1e9:.0f} ns/iter", flush=True)
    exec_ns = (times[4 + iters] - times[4]) / iters * 1e9
    print(f"bench: marginal {exec_ns:.0f} ns/iter", flush=True)

    results = [
        {name: np.asarray(out_arrs[i]).reshape(n_cores, *out_avals[i].shape)[c]
         for i, name in enumerate(out_names)}
        for c in range(n_cores)
    ]
    return results, exec_ns
